# revision 22
# baseline (speedup 1.0000x reference)
"""Trainium2 Bass kernel for nn_EncoderLayer_31825707664096.

Gemma-style encoder layer (RMSNorm + GQA attention w/ QK-norm + RoPE + GeGLU
MLP), batch=1, seq=2048, hidden=768, 3 heads x 256 head_dim, 1 KV head,
inter=1152, fp32.

Strategy: sequence-parallel over 8 cores (each core owns 256 query rows and
recomputes the full K/V — collectives measured ~120us under this harness, so
no cross-core traffic). All activations live feature-major ([feature, seq])
in SBUF.

v3 changes (vs the 158us v2):
- h, trig, the q/k/v projection weights AND the rope'd q/k activations are
  fp16 (vs f32r): halves phase-1 DMA (17MB -> 8.6MB) and the scores
  LDWEIGHTS time at ~0.05% relative error (bf16 tripled the error; fp16 is
  free). Output is bf16, converted on the host.
- all sqrt-based norm chains (qk-norms, post-attn, pre-ffn, post-ffn) are
  clustered outside the softmax-exp window: the compiler picks one
  activation-table set per function greedily, so interleaving sqrt with exp
  reloads tables (1.3us each) every switch.
- k-norm rstd multiplies kt (not the exp scale operand): scale-AP'd
  activations run at half rate, so the exps stay 512-wide bias-only.
- attention is head-sequential (one denominator row + one pn accumulator
  pair rotating) with the K/V slices produced ahead under the DMA stream.
- PSUM accumulation groups NEVER interleave within a bank (hardware
  corrupts them): Q-proj is kc-inner, AV uses one bank per d-chunk, the
  down-proj streams only one group per bank (even hc) during gate/up.
- MLP: the up-side pre-FFN rstd cancels against the scale-invariant
  post-FFN rmsnorm (a per-column factor survives the feature contraction),
  so only the gate input is scaled: 2 vector ops per chunk, no stall on
  the rstd chain; down-proj accumulates right behind gate/up.
- wo projection runs on the attention psum pools so no pool barrier (and
  no HAM re-throttle) before it; the barrier lands in the h2 bubble.
- 12 warmup matmuls during the DMA wait hold the PE HAM activity window
  busy so real matmuls start at 2.4GHz instead of 1.2GHz.
- DMA triggers ordered by first use, K slices streamed ahead of attention.

Per-core output is the feature-major [768, 256] shard; the host transposes
and concatenates.
"""

from contextlib import ExitStack

import ml_dtypes
import numpy as np

import concourse.mybir as mybir
from concourse import bass_isa
import concourse.tile as tile
from concourse import bacc
from concourse.bass_utils import run_bass_kernel_spmd

P = 128
S = 2048          # sequence length
H = 768           # hidden
D = 256           # head dim (also total KV width)
NH = 3            # query heads
I = 1152          # mlp intermediate
NC = 8            # cores
SL = S // NC      # 256 query rows per core
HC = H // P       # 6
DC = D // P       # 2
IC = I // P       # 9
KC = S // P       # 16 key chunks
NSL = S // 512    # 4 512-wide column slices
EPS = 1e-6
C_SHIFT = 30.0    # exp(s - C_SHIFT): keeps unnormalized softmax in fp32 range

F32 = mybir.dt.float32
F32R = mybir.dt.float32r
F16 = mybir.dt.float16
BF16 = mybir.dt.bfloat16
MUL = mybir.AluOpType.mult
AF = mybir.ActivationFunctionType

# small-pack column offsets
O_COSQ = 0            # [2*SL]
O_SINQ = 2 * SL       # [2*SL]
O_QW1 = 4 * SL        # [2]
O_KW1 = O_QW1 + 2     # [2]
O_WAW = O_KW1 + 2     # [6]
O_WFW = O_WAW + 6     # [6]
O_RIN = O_WFW + 6     # [16]
SMALL_W = O_RIN + 16

_CACHED = {}


def _build(debug=False):
    nc = bacc.Bacc("TRN2", target_bir_lowering=False, debug=False,
                   num_devices=NC)

    # ---- DRAM I/O (all host-packed in SBUF layout [p, chunk, cols]) ----
    small = nc.dram_tensor("small", [P, SMALL_W], F32, kind="ExternalInput").ap()
    wk = nc.dram_tensor("wk", [P, HC, D], F16, kind="ExternalInput").ap()
    htp = nc.dram_tensor("htp", [P, NSL, HC, 512], F16, kind="ExternalInput").ap()
    trig = nc.dram_tensor("trig", [P, NSL, 4, 512], F16, kind="ExternalInput").ap()
    hqf = nc.dram_tensor("hqf", [P, HC, SL], F16, kind="ExternalInput").ap()
    wv = nc.dram_tensor("wv", [P, HC, D], F16, kind="ExternalInput").ap()
    wq = nc.dram_tensor("wq", [P, HC, H], F16, kind="ExternalInput").ap()
    wo = nc.dram_tensor("wo", [P, HC, H], BF16, kind="ExternalInput").ap()
    wgu = nc.dram_tensor("wgu", [P, HC, 2 * I], BF16, kind="ExternalInput").ap()
    wd = nc.dram_tensor("wd", [P, IC, H], BF16, kind="ExternalInput").ap()
    outt = nc.dram_tensor("outt", [P, HC, SL], BF16, kind="ExternalOutput").ap()
    if debug:
        d_qt_bf = nc.dram_tensor("d_qt", [P, HC, SL], F16, kind="ExternalOutput").ap()
        d_kt_bf = nc.dram_tensor("d_kt", [P, DC, S], F16, kind="ExternalOutput").ap()
        d_v = nc.dram_tensor("d_v", [P, KC, D], BF16, kind="ExternalOutput").ap()
        d_at = nc.dram_tensor("d_at", [P, HC, SL], BF16, kind="ExternalOutput").ap()
        d_h2 = nc.dram_tensor("d_h2", [P, HC, SL], BF16, kind="ExternalOutput").ap()
        d_pq = nc.dram_tensor("d_pq", [P, HC, SL], F32, kind="ExternalOutput").ap()

    with tile.TileContext(nc) as tc:
        es = ExitStack()
        pp = es.enter_context(tc.tile_pool(name="persist", bufs=1))
        rot = es.enter_context(tc.tile_pool(name="rot", bufs=3))
        bcp = es.enter_context(tc.tile_pool(name="bcp", bufs=5))
        # attention-scoped pools (SBUF + all 8 PSUM banks)
        esa = ExitStack()
        kvp = esa.enter_context(tc.tile_pool(name="kvp", bufs=1))
        upool = esa.enter_context(tc.tile_pool(name="upool", bufs=2))
        trp = esa.enter_context(tc.tile_pool(name="trp", bufs=2))
        rot2 = esa.enter_context(tc.tile_pool(name="rot2", bufs=2))
        pbig = esa.enter_context(tc.tile_pool(name="pbig", bufs=5, space="PSUM"))
        pnp = esa.enter_context(tc.tile_pool(name="pnp", bufs=2, space="PSUM"))
        denp = esa.enter_context(tc.tile_pool(name="denp", bufs=1, space="PSUM"))

        # ======== DMA triggers in first-use order ==========================
        small_sb = pp.tile([P, SMALL_W], F32, tag="small")
        nc.sync.dma_start(small_sb[:], small)
        wk_sb = kvp.tile([P, HC, D], F16, tag="wk")
        nc.sync.dma_start(wk_sb[:], wk)
        ht_sb = kvp.tile([P, NSL, HC, 512], F16, tag="ht")
        nc.sync.dma_start(ht_sb[:, 0], htp[:, 0])
        trig_tiles = []
        tsl = trp.tile([P, 4, 512], F16, tag="trig", name="trig0")
        nc.sync.dma_start(tsl[:], trig[:, 0])
        trig_tiles.append(tsl)
        nc.sync.dma_start(ht_sb[:, 1], htp[:, 1])
        tsl = trp.tile([P, 4, 512], F16, tag="trig", name="trig1")
        nc.sync.dma_start(tsl[:], trig[:, 1])
        trig_tiles.append(tsl)
        hqf_sb = pp.tile([P, HC, SL], F16, tag="hqf")
        nc.sync.dma_start(hqf_sb[:], hqf)
        wv_sb = kvp.tile([P, HC, D], F16, tag="wv")
        nc.sync.dma_start(wv_sb[:], wv)
        wq_sb = kvp.tile([P, HC, H], F16, tag="wq")
        nc.sync.dma_start(wq_sb[:], wq)
        for sl in range(2, NSL):
            nc.sync.dma_start(ht_sb[:, sl], htp[:, sl])
            tsl = trp.tile([P, 4, 512], F16, tag="trig", name=f"trig{sl}")
            nc.sync.dma_start(tsl[:], trig[:, sl])
            trig_tiles.append(tsl)
        wo_sb = pp.tile([P, HC, H], BF16, tag="wo")
        nc.sync.dma_start(wo_sb[:], wo)
        wgu_sb = pp.tile([P, HC, 2 * I], BF16, tag="wgu")
        nc.sync.dma_start(wgu_sb[:], wgu)
        wd_sb = pp.tile([P, IC, H], BF16, tag="wd")
        nc.sync.dma_start(wd_sb[:], wd)

        # one softmax-denominator row, reused head-sequentially
        den_row = denp.tile([1, 512], F32, tag="den")

        # ======== constants + scratch =====================================
        ones_bf = pp.tile([P, 1], BF16, tag="ones")
        nc.vector.memset(ones_bf[:], 1.0)
        ones_f = pp.tile([P, 1], F32, tag="onesfr")
        nc.vector.memset(ones_f[:], 1.0)
        eps1 = pp.tile([1, 1], F32, tag="eps1")
        nc.vector.memset(eps1[:], EPS)
        eps_col = pp.tile([P, 1], F32, tag="epscol")
        nc.vector.memset(eps_col[:], EPS)
        biasC = pp.tile([P, 1], F32, tag="biasC")
        nc.vector.memset(biasC[:], -C_SHIFT)
        ones_h = pp.tile([P, 1], F16, tag="onesh")
        nc.vector.memset(ones_h[:], 1.0)
        warm = pp.tile([P, 512], BF16, tag="warm")
        nc.vector.memset(warm[:], 1.0)

        # prime the sqrt activation table set during the DMA wait
        prime_row = pp.tile([1, 1], F32, tag="prime", name="prime")
        nc.scalar.activation(prime_row[:], eps1[:], AF.Sqrt)

        # persistent activations
        dbg_pq = (pp.tile([P, HC, SL], F32, tag="dbgpq", name="dbgpq")
                  if debug else None)
        qt_f = pp.tile([P, HC, SL], F16, tag="qtf")
        kt_f = pp.tile([P, DC, S], F16, tag="ktf")
        v_sb = pp.tile([P, KC, D], BF16, tag="v")
        at_f = pp.tile([P, HC, SL], BF16, tag="atf")
        h2 = pp.tile([P, HC, SL], BF16, tag="h2")

        qw1 = small_sb[:, O_QW1:O_QW1 + 2]
        kw1 = small_sb[:, O_KW1:O_KW1 + 2]
        waw = small_sb[:, O_WAW:O_WAW + 6]
        wfw = small_sb[:, O_WFW:O_WFW + 6]
        rin = small_sb[:, O_RIN:O_RIN + 16]

        def cosq(dd):
            return small_sb[:, O_COSQ + dd * SL:O_COSQ + (dd + 1) * SL]

        def sinq(dd):
            return small_sb[:, O_SINQ + dd * SL:O_SINQ + (dd + 1) * SL]

        # ======== PE warmup: hold the HAM busy window during DMA wait ======
        def warm_burst(n, name):
            wp = pbig.tile([P, 512], F32, tag="mm", name=name)
            for w in range(n):
                nc.tensor.matmul(wp[0:1, :], ones_bf[:],
                                 warm[:], start=True, stop=True)

        warm_burst(12, "warmps")

        def rstd_bcast(in_row, scale, name):
            """[P,SL] broadcast of (scale*in + eps)^-0.5 (sqrt + reciprocal,
            keeping the scalar engine on the sqrt table set)."""
            srow = rot.tile([1, SL], F32, tag="lrow", name=name)
            nc.scalar.activation(srow[:], in_row, AF.Sqrt,
                                 bias=eps1[:], scale=scale)
            out_b = bcp.tile([P, SL], F32, tag="bcast", name=name + "b")
            nc.gpsimd.partition_broadcast(out_b[:], srow[:], channels=P)
            nc.vector.reciprocal_approx_fast(out=out_b[:], in_=out_b[:])
            return out_b

        # ======== K slice: proj + k-norm columns + rope ====================
        def k_slice(sl):
            sl_s = slice(sl * 512, (sl + 1) * 512)
            pk = [pbig.tile([P, 512], F32, tag="mm", name=f"pk{sl}_{d_}")
                  for d_ in range(DC)]
            for d in range(DC):
                for kc in range(HC):
                    nc.tensor.matmul(
                        pk[d][:], wk_sb[:, kc, d * P:(d + 1) * P],
                        ht_sb[:, sl, kc, :],
                        start=(kc == 0), stop=(kc == HC - 1))
            # squared projections for the norm
            sq5 = rot2.tile([P, 2, 512], F16, tag="sq5", name=f"ksq{sl}")
            for d in range(DC):
                nc.scalar.activation(sq5[:, d, :], pk[d][:], AF.Square)
            # k-norm rstd row -> broadcast -> reciprocal (kept off the
            # exp's scale operand: scale-AP'd activations run at half rate)
            ksp = pbig.tile([P, 512], F32, tag="mm", name=f"kss{sl}")
            for d in range(DC):
                nc.tensor.matmul(ksp[0:1, :], ones_h[:], sq5[:, d, :],
                                 start=(d == 0), stop=(d == DC - 1))
            ck_b = bcp.tile([P, 512], F32, tag="kbc", name=f"ckb{sl}")
            ckrow = rot.tile([1, 512], F32, tag="krow", name=f"ckr{sl}")
            nc.scalar.activation(ckrow[:], ksp[0:1, :], AF.Sqrt,
                                 bias=eps1[:], scale=1.0 / D)
            nc.gpsimd.partition_broadcast(ck_b[:], ckrow[:], channels=P)
            nc.vector.reciprocal_approx_fast(out=ck_b[:], in_=ck_b[:])
            tt = trig_tiles[sl]
            t0 = rot2.tile([P, 512], F32, tag="krA", name=f"krA{sl}")
            tb = rot2.tile([P, 512], F32, tag="krB", name=f"krB{sl}")
            nc.vector.scalar_tensor_tensor(
                t0[:], pk[0][:], kw1[:, 0:1], tt[:, 0, :], MUL, MUL)
            nc.vector.scalar_tensor_tensor(
                tb[:], pk[1][:], kw1[:, 1:2], tt[:, 2, :], MUL, MUL)
            nc.vector.tensor_sub(t0[:], t0[:], tb[:])
            nc.vector.tensor_mul(kt_f[:, 0, sl_s], t0[:], ck_b[:])
            t2 = rot2.tile([P, 512], F32, tag="krA", name=f"krC{sl}")
            t3 = rot2.tile([P, 512], F32, tag="krB", name=f"krD{sl}")
            nc.vector.scalar_tensor_tensor(
                t2[:], pk[1][:], kw1[:, 1:2], tt[:, 1, :], MUL, MUL)
            nc.vector.scalar_tensor_tensor(
                t3[:], pk[0][:], kw1[:, 0:1], tt[:, 3, :], MUL, MUL)
            nc.vector.tensor_add(t2[:], t2[:], t3[:])
            nc.vector.tensor_mul(kt_f[:, 1, sl_s], t2[:], ck_b[:])

        # ======== V slice: 4 chunks in 2 psum tiles ========================
        def v_slice(sl):
            for half in range(2):
                pv = pbig.tile([P, 2, D], F32, tag="mm", name=f"pv{sl}_{half}")
                for j in range(2):
                    col = (2 * half + j) * P
                    for kc in range(HC):
                        nc.tensor.matmul(
                            pv[:, j, :], ht_sb[:, sl, kc, col:col + P],
                            wv_sb[:, kc, :],
                            start=(kc == 0), stop=(kc == HC - 1))
                for j in range(2):
                    sc = 4 * sl + 2 * half + j
                    nc.scalar.mul(v_sb[:, sc, :], pv[:, j, :],
                                  rin[:, sc:sc + 1])

        # ======== Q: proj streamed per wq chunk, then norm + rope ==========
        def q_proj():
            pq = [pbig.tile([P, 2, SL], F32, tag="mm", name=f"pq{h}")
                  for h in range(NH)]
            for h in range(NH):
                for d in range(DC):
                    oc = 2 * h + d
                    for kc in range(HC):
                        nc.tensor.matmul(
                            pq[h][:, d, :], wq_sb[:, kc, oc * P:(oc + 1) * P],
                            hqf_sb[:, kc, :],
                            start=(kc == 0), stop=(kc == HC - 1))
            return pq

        def q_post(pq, h):
            sqq = rot.tile([P, 2, SL], F32R, tag="sqq", name=f"qsq{h}")
            nc.scalar.activation(
                sqq[:].rearrange("p a b -> p (a b)"),
                pq[h][:].rearrange("p a b -> p (a b)"), AF.Square)
            if debug:
                for d in range(DC):
                    nc.scalar.copy(dbg_pq[:, 2 * h + d, :], pq[h][:, d, :])
            qsp = pbig.tile([P, 512], F32, tag="mm", name=f"qss{h}")
            for d in range(DC):
                nc.tensor.matmul(qsp[0:1, 0:SL],
                                 ones_f[:].bitcast(F32R), sqq[:, d, :],
                                 start=(d == 0), stop=(d == DC - 1))
            rq_b = rstd_bcast(qsp[0:1, 0:SL], 1.0 / D, f"rq{h}")
            t0 = rot.tile([P, SL], F32, tag="rA", name=f"rA{h}")
            tb = rot.tile([P, SL], F32, tag="rB", name=f"rB{h}")
            nc.vector.scalar_tensor_tensor(
                t0[:], pq[h][:, 0, :], qw1[:, 0:1], cosq(0), MUL, MUL)
            nc.vector.scalar_tensor_tensor(
                tb[:], pq[h][:, 1, :], qw1[:, 1:2], sinq(0), MUL, MUL)
            nc.vector.tensor_sub(t0[:], t0[:], tb[:])
            nc.vector.tensor_mul(qt_f[:, 2 * h, :], t0[:], rq_b[:])
            t2 = rot.tile([P, SL], F32, tag="rA", name=f"rC{h}")
            t3 = rot.tile([P, SL], F32, tag="rB", name=f"rD{h}")
            nc.vector.scalar_tensor_tensor(
                t2[:], pq[h][:, 1, :], qw1[:, 1:2], cosq(1), MUL, MUL)
            nc.vector.scalar_tensor_tensor(
                t3[:], pq[h][:, 0, :], qw1[:, 0:1], sinq(1), MUL, MUL)
            nc.vector.tensor_add(t2[:], t2[:], t3[:])
            nc.vector.tensor_mul(qt_f[:, 2 * h + 1, :], t2[:], rq_b[:])

        # ======== scores + exp + den + AV for one slice ====================
        def scores_slice(sl, h, u_sb):
            for pair in range(2):
                sp = pbig.tile([P, 2, SL], F32, tag="mm",
                               name=f"sp{sl}_{h}_{pair}")
                for j in range(2):
                    ksc = 4 * sl + 2 * pair + j
                    for d in range(DC):
                        nc.tensor.matmul(
                            sp[:, j, :],
                            kt_f[:, d, ksc * P:(ksc + 1) * P],
                            qt_f[:, 2 * h + d, :],
                            start=(d == 0), stop=(d == DC - 1))
                ksc0 = 4 * sl + 2 * pair
                nc.scalar.activation(
                    u_sb[:, ksc0:ksc0 + 2, :].rearrange("p a b -> p (a b)"),
                    sp[:].rearrange("p a b -> p (a b)"),
                    AF.Exp, bias=biasC[:])
                for j in range(2):
                    ksc = ksc0 + j
                    nc.tensor.matmul(den_row[0:1, 0:SL], ones_bf[:],
                                     u_sb[:, ksc, :],
                                     start=(ksc == 0), stop=(ksc == KC - 1))

        def av_slice(sl, h, u_sb, pn_t):
            # pn_t is a pair of full-bank tiles: PSUM accumulation groups
            # must not interleave within one bank, so each d-chunk gets its
            # own bank and only cross-bank interleaving remains.
            for d in range(DC):
                for kk in range(4):
                    ksc = 4 * sl + kk
                    nc.tensor.matmul(
                        pn_t[d][:, 0, :], v_sb[:, ksc, d * P:(d + 1) * P],
                        u_sb[:, ksc, :],
                        start=(ksc == 0), stop=(ksc == KC - 1))

        def finish_head(h, u_sb, pn_t):
            drow = rot.tile([1, SL], F32, tag="row", name=f"drow{h}")
            nc.scalar.copy(drow[:], den_row[0:1, 0:SL])
            den_b = bcp.tile([P, SL], F32, tag="bcast", name=f"denb{h}")
            nc.gpsimd.partition_broadcast(den_b[:], drow[:], channels=P)
            nc.vector.reciprocal_approx_fast(out=den_b[:], in_=den_b[:])
            for d in range(DC):
                nc.vector.tensor_mul(at_f[:, 2 * h + d, :], pn_t[d][:, 0, :],
                                     den_b[:])

        # ======== attention schedule ======================================
        # All projections and sqrt-based norm chains run before the first
        # softmax exp so the scalar engine loads each activation table once
        # (sqrt -> exp -> sqrt -> gelu -> sqrt). Heads run sequentially so a
        # single denominator row / pn accumulator pair rotates cleanly.
        k_slice(0)
        k_slice(1)
        pq = q_proj()
        q_post(pq, 0)
        q_post(pq, 1)
        q_post(pq, 2)
        v_slice(0)
        v_slice(1)
        for sl in range(2, NSL):
            k_slice(sl)
            v_slice(sl)
        for h in range(NH):
            u_t = upool.tile([P, KC, SL], BF16, tag="u", name=f"u{h}")
            pn_t = [pnp.tile([P, 2, SL], F32, tag="pn", name=f"pn{h}_{d_}")
                    for d_ in range(DC)]
            for sl in range(NSL):
                scores_slice(sl, h, u_t)
                av_slice(sl, h, u_t, pn_t)
            finish_head(h, u_t, pn_t)

        # ======== wo projection + post-attn norm + residual ================
        # runs on the attention pools (pbig psum + den_row for the sqsum row)
        # so there is no psum-pool barrier before it; the barrier lands after
        # the pre-FFN norm, overlapped with the h2 dependency bubble.
        pw3 = [pbig.tile([P, 2, SL], F32, tag="mm", name=f"pwo{i_}")
               for i_ in range(HC // 2)]
        pp6 = [pw3[i_ // 2][:, i_ % 2, :] for i_ in range(HC)]
        for hc in range(HC):
            for oc in range(HC):
                nc.tensor.matmul(
                    pp6[hc], wo_sb[:, oc, hc * P:(hc + 1) * P],
                    at_f[:, oc, :],
                    start=(oc == 0), stop=(oc == HC - 1))
            sq = rot.tile([P, SL], F32R, tag="sq", name=f"psq{hc}")
            nc.scalar.activation(sq[:], pp6[hc], AF.Square)
            nc.tensor.matmul(den_row[0:1, 0:SL], ones_f[:].bitcast(F32R),
                             sq[:], start=(hc == 0), stop=(hc == HC - 1))
        ra_b = rstd_bcast(den_row[0:1, 0:SL], 1.0 / H, "ra")
        for hc in range(HC):
            t = rot.tile([P, SL], F32, tag="rA", name=f"wot{hc}")
            nc.vector.scalar_tensor_tensor(
                t[:], pp6[hc], waw[:, hc:hc + 1], ra_b[:], MUL, MUL)
            nc.vector.tensor_add(h2[:, hc, :], t[:], hqf_sb[:, hc, :])

        # ======== pre-FFN norm (gate side only; up side cancels) ===========
        for pr in range(HC // 2):
            sq = rot.tile([P, 2, SL], F32R, tag="sqq", name=f"fsq{pr}")
            nc.scalar.activation(
                sq[:].rearrange("p a b -> p (a b)"),
                h2[:, 2 * pr:2 * pr + 2, :].rearrange("p a b -> p (a b)"),
                AF.Square)
            for j in range(2):
                nc.tensor.matmul(den_row[0:1, 0:SL], ones_f[:].bitcast(F32R),
                                 sq[:, j, :], start=(pr == 0 and j == 0),
                                 stop=(pr == HC // 2 - 1 and j == 1))
        r2_b = rstd_bcast(den_row[0:1, 0:SL], 1.0 / H, "r2")

        esa.close()  # free ht/trig/wk/wv/wq/u + all 8 PSUM banks
        es2 = ExitStack()
        prow = es2.enter_context(tc.tile_pool(name="prow", bufs=1,
                                              space="PSUM"))
        mlp_sb = es2.enter_context(tc.tile_pool(name="mlp_sb", bufs=1))
        rows = prow.tile([1, 512], F32, tag="rows")
        act_all = mlp_sb.tile([P, IC, SL], BF16, tag="actall")
        out_sb = mlp_sb.tile([P, HC, SL], BF16, tag="outsb")

        # ======== MLP: gate/up + interleaved down-proj =====================
        with tc.tile_pool(name="pd6", bufs=1, space="PSUM") as pd6, \
             tc.tile_pool(name="pgu", bufs=2, space="PSUM") as pgu:
            pd3 = [pd6.tile([P, 2, SL], F32, tag=f"pm{i_}", name=f"pm{i_}")
                   for i_ in range(HC // 2)]
            pm6 = [pd3[i_ // 2][:, i_ % 2, :] for i_ in range(HC)]

            def gate_up(ic):
                pg = pgu.tile([P, 2, SL], F32, tag="gu", name=f"pg{ic}")
                for kc in range(HC):
                    nc.tensor.matmul(
                        pg[:, 0, :], wgu_sb[:, kc, ic * P:(ic + 1) * P],
                        h2[:, kc, :],
                        start=(kc == 0), stop=(kc == HC - 1))
                for kc in range(HC):
                    nc.tensor.matmul(
                        pg[:, 1, :], wgu_sb[:, kc, I + ic * P:I + (ic + 1) * P],
                        h2[:, kc, :],
                        start=(kc == 0), stop=(kc == HC - 1))
                gt = rot.tile([P, SL], BF16, tag="gt", name=f"gt{ic}")
                nc.vector.tensor_mul(gt[:], pg[:, 0, :], r2_b[:])
                gl = rot.tile([P, SL], BF16, tag="gl", name=f"gl{ic}")
                nc.scalar.activation(gl[:], gt[:], AF.Gelu_apprx_tanh)
                nc.vector.tensor_mul(act_all[:, ic, :], gl[:], pg[:, 1, :])

            def down_even(ic):
                # one open accumulation group per bank (hc 0/2/4); the odd
                # hc groups run densely afterwards so no bank ever holds two
                # interleaved groups.
                for hc in (0, 2, 4):
                    nc.tensor.matmul(
                        pm6[hc], wd_sb[:, ic, hc * P:(hc + 1) * P],
                        act_all[:, ic, :],
                        start=(ic == 0), stop=(ic == IC - 1))

            gate_up(0)
            for ic in range(1, IC):
                gate_up(ic)
                down_even(ic - 1)
            down_even(IC - 1)
            for pr, hc in enumerate((1, 3, 5)):
                for ic in range(IC):
                    nc.tensor.matmul(
                        pm6[hc], wd_sb[:, ic, hc * P:(hc + 1) * P],
                        act_all[:, ic, :],
                        start=(ic == 0), stop=(ic == IC - 1))
                # pair pr is complete once its odd half stops; its square-sum
                # runs behind the next pair's down matmuls
                sq = rot.tile([P, 2, SL], F32R, tag="sqq", name=f"msq{pr}")
                nc.scalar.activation(
                    sq[:].rearrange("p a b -> p (a b)"),
                    pd3[pr][:].rearrange("p a b -> p (a b)"), AF.Square)
                for j in range(2):
                    nc.tensor.matmul(rows[:, 0:SL],
                                     ones_f[:].bitcast(F32R), sq[:, j, :],
                                     start=(pr == 0 and j == 0),
                                     stop=(pr == HC // 2 - 1 and j == 1))
            r3_b = rstd_bcast(rows[:, 0:SL], 1.0 / H, "r3")
            for hc in range(HC):
                t = rot.tile([P, SL], F32, tag="rA", name=f"mt{hc}")
                nc.vector.scalar_tensor_tensor(
                    t[:], pm6[hc], wfw[:, hc:hc + 1], r3_b[:], MUL, MUL)
                nc.vector.tensor_add(out_sb[:, hc, :], t[:], h2[:, hc, :])
                if hc % 2 == 1:
                    nc.sync.dma_start(outt[:, hc - 1:hc + 1, :],
                                      out_sb[:, hc - 1:hc + 1, :])

        if debug:
            nc.sync.dma_start(d_qt_bf, qt_f[:])
            nc.sync.dma_start(d_kt_bf, kt_f[:])
            nc.sync.dma_start(d_v, v_sb[:])
            nc.sync.dma_start(d_at, at_f[:])
            nc.sync.dma_start(d_h2, h2[:])
            nc.sync.dma_start(d_pq, dbg_pq[:])
        es2.close()
        es.close()

    nc.compile()
    return nc


def _get_nc(debug=False):
    key = ("ncd" if debug else "nc")
    if key not in _CACHED:
        _CACHED[key] = _build(debug)
    return _CACHED[key]


def _pack(a, c, p=P):
    """[c*p, X] row-major -> [p, c, X]."""
    return np.ascontiguousarray(
        a.reshape(c, p, *a.shape[1:]).transpose(1, 0, 2))


def _prep_inputs(hidden_states, cos, sin, wq, wk, wv, wo, q_norm_w, k_norm_w,
                 ln_in_w, ln_post_attn_w, ln_pre_ffn_w, ln_post_ffn_w,
                 wg, wu, wd):
    f = np.float32
    f16 = np.float16
    bf = ml_dtypes.bfloat16
    ct = np.ascontiguousarray

    hid = np.asarray(hidden_states, f)[0]            # [S, H]
    hT = ct(hid.T)                                   # [H, S]
    cosT = ct(np.asarray(cos, f)[0, 0].T)            # [D, S]
    sinT = ct(np.asarray(sin, f)[0, 0].T)

    g_in = 1.0 + np.asarray(ln_in_w, f)
    g_ffn = 1.0 + np.asarray(ln_pre_ffn_w, f)

    # host-side input-RMSNorm rstd (V scale; Q/K absorb it into qk-norm)
    rin_full = 1.0 / np.sqrt((hT * hT).mean(axis=0) + EPS)          # [S]

    wgut = np.concatenate(
        [(np.asarray(wg, f) * g_ffn[None, :]).T,
         (np.asarray(wu, f) * g_ffn[None, :]).T], axis=1)           # [H, 2I]

    ht_pack = _pack(hT, HC)                                         # [P,HC,S]
    htq = np.ascontiguousarray(
        ht_pack.reshape(P, HC, NSL, 512).transpose(0, 2, 1, 3))     # [P,4,HC,512]
    trig_pack = np.concatenate([_pack(cosT, DC), _pack(sinT, DC)],
                               axis=1)                              # [P,4,S]
    trigq = np.ascontiguousarray(
        trig_pack.reshape(P, 4, NSL, 512).transpose(0, 2, 1, 3))    # [P,4,4,512]

    shared = {
        "wq": _pack((np.asarray(wq, f) * g_in[None, :]).T, HC).astype(f16),
        "htp": htq.astype(f16),
        "trig": trigq.astype(f16),
        "wk": _pack((np.asarray(wk, f) * g_in[None, :]).T, HC).astype(f16),
        "wv": _pack((np.asarray(wv, f) * g_in[None, :]).T, HC).astype(f16),
        "wo": _pack(np.asarray(wo, f).T, HC).astype(bf),
        "wgu": _pack(wgut, HC).astype(bf),
        "wd": _pack(np.asarray(wd, f).T, IC).astype(bf),
    }
    cos_pack = _pack(cosT, DC)                                      # [P,DC,S]
    sin_pack = _pack(sinT, DC)
    qw1 = (1.0 + np.asarray(q_norm_w, f)).reshape(DC, P).T          # [P,2]
    kw1 = (1.0 + np.asarray(k_norm_w, f)).reshape(DC, P).T
    waw = (1.0 + np.asarray(ln_post_attn_w, f)).reshape(HC, P).T    # [P,6]
    wfw = (1.0 + np.asarray(ln_post_ffn_w, f)).reshape(HC, P).T
    rin_col = rin_full.reshape(KC, P).T                             # [P,16]

    in_maps = []
    for c in range(NC):
        cols = slice(c * SL, (c + 1) * SL)
        small = np.empty((P, SMALL_W), f)
        small[:, O_COSQ:O_COSQ + 2 * SL] = \
            cos_pack[:, :, cols].reshape(P, 2 * SL)
        small[:, O_SINQ:O_SINQ + 2 * SL] = \
            sin_pack[:, :, cols].reshape(P, 2 * SL)
        small[:, O_QW1:O_QW1 + 2] = qw1
        small[:, O_KW1:O_KW1 + 2] = kw1
        small[:, O_WAW:O_WAW + 6] = waw
        small[:, O_WFW:O_WFW + 6] = wfw
        small[:, O_RIN:O_RIN + 16] = rin_col
        m = dict(shared)
        m["small"] = small
        m["hqf"] = _pack(hT[:, cols], HC).astype(f16)
        in_maps.append(m)
    return in_maps


def run(trace=False, tmpdir=None, debug=False, **inputs):
    """Build (cached), run on 8 cores, reassemble. Returns (output, results)."""
    nc = _get_nc(debug)
    in_maps = _prep_inputs(
        inputs["hidden_states"], inputs["cos"], inputs["sin"],
        inputs["wq"], inputs["wk"], inputs["wv"], inputs["wo"],
        inputs["q_norm_w"], inputs["k_norm_w"],
        inputs["ln_in_w"], inputs["ln_post_attn_w"],
        inputs["ln_pre_ffn_w"], inputs["ln_post_ffn_w"],
        inputs["wg"], inputs["wu"], inputs["wd"],
    )
    res = run_bass_kernel_spmd(nc, in_maps, list(range(NC)),
                               trace=trace, tmpdir=tmpdir)
    out = np.empty((S, H), np.float32)
    for c in range(NC):
        o = res.results[c]["outt"].astype(np.float32)  # [P, HC, SL]
        out[c * SL:(c + 1) * SL, :] = \
            o.transpose(1, 0, 2).reshape(H, SL).T
    return out[None], res


def kernel(**inputs):
    out, _ = run(trace=False, **inputs)
    return out
